# revision 30
# baseline (speedup 1.0000x reference)
"""CBOW negative-sampling-style loss kernel for trn2, 8 NeuronCores.

Sharding: pure batch data-parallel. Each core owns 256 batch rows
(2 tiles of 128): it gathers emb_v for its rows, builds h, and computes
the FULL-vocab negative path for those rows by streaming the whole
transposed emb_u table (host-relaid [100, 50000] f32) through SBUF in
bf16. Per-row work: scores = hT_own^T @ uT, S_b = sum_v sigmoid(-score)
fused on ScalarE (accum_out), positive path sd_b = sigmoid(dot(emb_u[y],
h)). Each core reduces ln(S_b) - ln(sd_b) over its rows to one scalar;
The per-core partial sums are returned and combined on the host (the
unshard step) -> zero collectives, no inter-core skew stalls.
"""

import os
import numpy as np

import concourse.bass as bass
import concourse.bacc as bacc
import concourse.mybir as mybir
import concourse.tile as tile
from concourse.bass_utils import run_bass_kernel_spmd

N_CORES = 8
V, E, B, CTX = 50000, 100, 2048, 10
EP = 128
BS = B // N_CORES     # 256 batch rows per core
P = 128
NT = BS // P          # 2 own batch tiles
GROUP = 2048          # PSUM span per ScalarE sigmoid call (4 banks)
NFULL = V // GROUP    # 24 full groups
TAIL = V - NFULL * GROUP   # 848
NG = NFULL + (1 if TAIL else 0)
CHUNK = 4096          # ut streaming chunk (f32 staging -> bf16 cast)
NPRE = 2              # chunks prefetched early on the scalar ring
MMN = 512             # matmul moving free dim (one PSUM bank)

F32 = mybir.dt.float32
BF16 = mybir.dt.bfloat16
I32 = mybir.dt.int32

_last_results = None  # test harness reads exec_time_ns off this


def _build():
    nc = bacc.Bacc("TRN2", target_bir_lowering=False, debug=False,
                   num_devices=N_CORES)

    x_in = nc.dram_tensor("x", [BS, CTX], I32, kind="ExternalInput").ap()
    y_in = nc.dram_tensor("y", [BS, 1], I32, kind="ExternalInput").ap()
    embv = nc.dram_tensor("emb_v", [V, E], F32, kind="ExternalInput").ap()
    embu = nc.dram_tensor("emb_u", [V, E], F32, kind="ExternalInput").ap()
    ut_in = nc.dram_tensor("ut", [E, V], F32, kind="ExternalInput").ap()
    loss_out = nc.dram_tensor("loss", [1, 1], F32, kind="ExternalOutput").ap()

    with tile.TileContext(nc) as tc:
        with tc.tile_pool(name="dram", bufs=1, space="DRAM") as dram, \
             tc.tile_pool(name="sbuf", bufs=1) as sb, \
             tc.tile_pool(name="gp", bufs=3) as gp, \
             tc.tile_pool(name="gat", bufs=24) as gat, \
             tc.tile_pool(name="stg", bufs=3) as stg:

            # x/y first on the sync HWDGE ring so gathers start immediately
            x_t = sb.tile([P, CTX * NT], I32)
            y_t = sb.tile([P, NT], I32)
            for t in range(NT):
                nc.sync.dma_start(out=x_t[:, t * CTX:(t + 1) * CTX],
                                  in_=x_in[t * P:(t + 1) * P, :])
                nc.sync.dma_start(out=y_t[:, t:t + 1],
                                  in_=y_in[t * P:(t + 1) * P, :])

            # stream full uT: DRAM f32 chunks -> SBUF staging -> bf16 cast.
            # The first 3 chunk loads go on the scalar HWDGE ring now (ACT is
            # idle early); the rest are emitted later on the sync ring so
            # their descriptors/data don't starve the gather DMAs or delay
            # h_own/transpose. ALL casts are emitted after the gather adds so
            # the in-order DVE queue isn't blocked behind 20MB of DMA.
            ut_b = sb.tile([E, V], BF16)
            chunks = []
            c0 = 0
            while c0 < V:
                cn = min(CHUNK, V - c0)
                chunks.append((c0, cn))
                c0 += cn
            stages = {}
            for (c0, cn) in chunks[:NPRE]:
                stage = stg.tile([E, CHUNK], F32, tag="stage")
                nc.scalar.dma_start(out=stage[:, :cn],
                                    in_=ut_in[:, c0:c0 + cn])
                stages[c0] = stage

            hT = sb.tile([EP, BS], BF16)
            sd = sb.tile([P, NT], F32)   # sigmoid(pos dot)

            # --- per-tile: gathers -> h -> hT column block ---
            hsums = []
            for t in range(NT):
                hsum = gp.tile([P, E], F32, tag="hsum")
                for c in range(CTX):
                    g = gat.tile([P, E], F32, tag="gather")
                    nc.gpsimd.indirect_dma_start(
                        out=g[:], out_offset=None, in_=embv[:],
                        in_offset=bass.IndirectOffsetOnAxis(
                            ap=x_t[:, t * CTX + c: t * CTX + c + 1], axis=0))
                    if c == 0:
                        nc.vector.tensor_copy(hsum[:], g[:])
                    else:
                        nc.vector.tensor_add(hsum[:], hsum[:], g[:])
                nc.vector.tensor_scalar_mul(hsum[:], hsum[:], 1.0 / CTX)
                hsums.append(hsum)

                # bf16 cast, then transpose into hT via DVE 32x32 block
                # transposes (the xbar DMA-transpose serializes against all
                # in-flight DMA traffic, stalling ~18us; DVE doesn't)
                hbf = gp.tile([P, EP], BF16, tag="hbf")
                nc.vector.memset(hbf[:, E:EP], 0.0)
                nc.vector.tensor_copy(hbf[:, :E], hsum[:])
                for bi in range(P // 32):
                    for bj in range(EP // 32):
                        nc.vector.transpose(
                            out=hT[bj * 32:(bj + 1) * 32,
                                   t * P + bi * 32: t * P + (bi + 1) * 32],
                            in_=hbf[bi * 32:(bi + 1) * 32,
                                    bj * 32:(bj + 1) * 32])
                early = chunks[:1] if t == 0 else chunks[1:NPRE]
                for (c0, cn) in early:
                    nc.vector.tensor_copy(ut_b[:, c0:c0 + cn],
                                          stages[c0][:, :cn])

            # positive path dots (sd ACTIVATEs deferred to the end so they
            # don't block the ScalarE queue mid-kernel)
            dfull = sb.tile([P, NT], F32)
            for t in range(NT):
                uy = gat.tile([P, E], F32, tag="gather")
                nc.gpsimd.indirect_dma_start(
                    out=uy[:], out_offset=None, in_=embu[:],
                    in_offset=bass.IndirectOffsetOnAxis(
                        ap=y_t[:, t:t + 1], axis=0))
                prod = gp.tile([P, E], F32, tag="prod")
                nc.vector.tensor_mul(prod[:], uy[:], hsums[t][:])
                nc.vector.tensor_reduce(dfull[:, t:t + 1], prod[:],
                                        axis=mybir.AxisListType.X,
                                        op=mybir.AluOpType.add)

            # remaining ut chunks: SWDGE cast-during-DMA straight into the
            # bf16 table (halves the SBUF-write bytes; no staging, no DVE
            # casts). On the gpsimd queue these naturally wait out the
            # gathers, so they don't steal gather bandwidth.
            for (c0, cn) in chunks[NPRE:]:
                nc.gpsimd.dma_start(out=ut_b[:, c0:c0 + cn],
                                    in_=ut_in[:, c0:c0 + cn])

            # --- main loop: scores -> sigmoid -> per-row full-vocab sums ---
            S_part = sb.tile([P, NT], F32)
            sig_scr = sb.tile([P, GROUP], BF16)
            groups = [(i * GROUP, GROUP) for i in range(NFULL)]
            if TAIL:
                groups.append((NFULL * GROUP, TAIL))
            accs = [sb.tile([P, NG], F32, tag=f"acc{t}", name=f"acc{t}")
                    for t in range(NT)]
            # group-major order so each ut chunk is consumed by both batch
            # tiles back-to-back (halves the required stream bandwidth).
            # Tile 1's first two groups are deferred until its hT is ready.
            T0F = 8   # tile-0 groups run solo while tile-1's hT finishes
            sched = [(g, 0) for g in range(T0F)]
            sched += [x for g in range(T0F, NG)
                      for x in ((g, 0), (g - T0F, 1))]
            sched += [(g, 1) for g in range(NG - T0F, NG)]
            with tc.tile_pool(name="mm_psum", bufs=2, space="PSUM") as mmp:
                for (gi, t) in sched:
                    v0, vn = groups[gi]
                    lhsT = hT[:E, t * P:(t + 1) * P]
                    pg = mmp.tile([P, GROUP], F32)
                    for n0 in range(0, vn, MMN):
                        nn = min(MMN, vn - n0)
                        nc.tensor.matmul(
                            pg[:, n0:n0 + nn], lhsT,
                            ut_b[:, v0 + n0: v0 + n0 + nn],
                            start=True, stop=True)
                    nc.scalar.activation(
                        sig_scr[:, :vn], pg[:, :vn],
                        mybir.ActivationFunctionType.Sigmoid,
                        scale=-1.0, accum_out=accs[t][:, gi:gi + 1])
                # deferred positive-path sigmoids
                for t in range(NT):
                    nc.scalar.activation(sd[:, t:t + 1], dfull[:, t:t + 1],
                                         mybir.ActivationFunctionType.Sigmoid)
                for t in range(NT):
                    nc.vector.tensor_reduce(S_part[:, t:t + 1], accs[t][:],
                                            axis=mybir.AxisListType.X,
                                            op=mybir.AluOpType.add)

            # --- final: partial = sum_own_b ln(S_b / sd_b); AllReduce ---
            Gr = sb.tile([P, NT], F32)
            nc.vector.reciprocal(Gr[:], sd[:])
            R = sb.tile([P, NT], F32)
            nc.vector.tensor_mul(R[:], S_part[:], Gr[:])
            L = sb.tile([P, NT], F32)
            nc.scalar.activation(L[:], R[:], mybir.ActivationFunctionType.Ln)
            Lr = sb.tile([P, 1], F32)
            nc.vector.tensor_reduce(Lr[:], L[:], axis=mybir.AxisListType.X,
                                    op=mybir.AluOpType.add)
            ones = sb.tile([P, 1], F32)
            nc.vector.memset(ones[:], 1.0)
            with tc.tile_pool(name="fin_psum", bufs=1, space="PSUM") as fpp:
                lp = fpp.tile([1, 1], F32)
                nc.tensor.matmul(lp[:], ones[:], Lr[:], start=True, stop=True)
                ls = sb.tile([1, 1], F32)
                nc.scalar.mul(ls[:], lp[:], 1.0 / B)
                nc.sync.dma_start(out=loss_out[:], in_=ls[:])

    nc.compile()
    return nc


_nc_cache = None


def kernel(x_positive, y, emb_v, emb_u):
    global _nc_cache, _last_results
    x32 = np.ascontiguousarray(np.asarray(x_positive, dtype=np.int32))
    y32 = np.ascontiguousarray(np.asarray(y, dtype=np.int32)).reshape(B, 1)
    ev = np.ascontiguousarray(np.asarray(emb_v, dtype=np.float32))
    eu = np.ascontiguousarray(np.asarray(emb_u, dtype=np.float32))
    ut = np.ascontiguousarray(eu.T)

    if _nc_cache is None:
        _nc_cache = _build()
    nc = _nc_cache

    in_maps = []
    for c in range(N_CORES):
        in_maps.append({
            "x": x32[c * BS:(c + 1) * BS, :],
            "y": y32[c * BS:(c + 1) * BS, :],
            "emb_v": ev,
            "emb_u": eu,
            "ut": ut,
        })

    trace = bool(os.environ.get("BASS_TRACE"))
    res = run_bass_kernel_spmd(nc, in_maps, list(range(N_CORES)), trace=trace)
    _last_results = res
    loss = np.float32(sum(res.results[c]["loss"][0, 0]
                          for c in range(N_CORES)))
    return np.asarray(loss, dtype=np.float32).reshape(())


# revision 31
# speedup vs baseline: 1.0001x; 1.0001x over previous
"""CBOW negative-sampling-style loss kernel for trn2, 8 NeuronCores.

Sharding: pure batch data-parallel. Each core owns 256 batch rows
(2 tiles of 128): it gathers emb_v for its rows, builds h, and computes
the FULL-vocab negative path for those rows by streaming the whole
transposed emb_u table (host-relaid [100, 50000] f32) through SBUF in
bf16. Per-row work: scores = hT_own^T @ uT, S_b = sum_v sigmoid(-score)
fused on ScalarE (accum_out), positive path sd_b = sigmoid(dot(emb_u[y],
h)). Each core reduces ln(S_b) - ln(sd_b) over its rows to one scalar;
The per-core partial sums are returned and combined on the host (the
unshard step) -> zero collectives, no inter-core skew stalls.
"""

import os
import numpy as np

import concourse.bass as bass
import concourse.bacc as bacc
import concourse.mybir as mybir
import concourse.tile as tile
from concourse.bass_utils import run_bass_kernel_spmd

N_CORES = 8
V, E, B, CTX = 50000, 100, 2048, 10
EP = 128
BS = B // N_CORES     # 256 batch rows per core
P = 128
NT = BS // P          # 2 own batch tiles
GROUP = 2048          # PSUM span per ScalarE sigmoid call (4 banks)
NFULL = V // GROUP    # 24 full groups
TAIL = V - NFULL * GROUP   # 848
NG = NFULL + (1 if TAIL else 0)
CHUNK = 4096          # ut streaming chunk (f32 staging -> bf16 cast)
NPRE = 2              # chunks prefetched early on the scalar ring
MMN = 512             # matmul moving free dim (one PSUM bank)

F32 = mybir.dt.float32
BF16 = mybir.dt.bfloat16
I32 = mybir.dt.int32

_last_results = None  # test harness reads exec_time_ns off this


def _build():
    nc = bacc.Bacc("TRN2", target_bir_lowering=False, debug=False,
                   num_devices=N_CORES)

    x_in = nc.dram_tensor("x", [BS, CTX], I32, kind="ExternalInput").ap()
    y_in = nc.dram_tensor("y", [BS, 1], I32, kind="ExternalInput").ap()
    embv = nc.dram_tensor("emb_v", [V, E], F32, kind="ExternalInput").ap()
    embu = nc.dram_tensor("emb_u", [V, E], F32, kind="ExternalInput").ap()
    ut_in = nc.dram_tensor("ut", [E, V], F32, kind="ExternalInput").ap()
    loss_out = nc.dram_tensor("loss", [1, 1], F32, kind="ExternalOutput").ap()

    with tile.TileContext(nc) as tc:
        with tc.tile_pool(name="sbuf", bufs=1) as sb, \
             tc.tile_pool(name="gp", bufs=3) as gp, \
             tc.tile_pool(name="gat", bufs=24) as gat, \
             tc.tile_pool(name="stg", bufs=3) as stg:

            # x/y first on the sync HWDGE ring so gathers start immediately
            x_t = sb.tile([P, CTX * NT], I32)
            y_t = sb.tile([P, NT], I32)
            for t in range(NT):
                nc.sync.dma_start(out=x_t[:, t * CTX:(t + 1) * CTX],
                                  in_=x_in[t * P:(t + 1) * P, :])
                nc.sync.dma_start(out=y_t[:, t:t + 1],
                                  in_=y_in[t * P:(t + 1) * P, :])

            # stream full uT: DRAM f32 chunks -> SBUF staging -> bf16 cast.
            # The first 3 chunk loads go on the scalar HWDGE ring now (ACT is
            # idle early); the rest are emitted later on the sync ring so
            # their descriptors/data don't starve the gather DMAs or delay
            # h_own/transpose. ALL casts are emitted after the gather adds so
            # the in-order DVE queue isn't blocked behind 20MB of DMA.
            ut_b = sb.tile([E, V], BF16)
            chunks = []
            c0 = 0
            while c0 < V:
                cn = min(CHUNK, V - c0)
                chunks.append((c0, cn))
                c0 += cn
            stages = {}
            for (c0, cn) in chunks[:NPRE]:
                stage = stg.tile([E, CHUNK], F32, tag="stage")
                nc.scalar.dma_start(out=stage[:, :cn],
                                    in_=ut_in[:, c0:c0 + cn])
                stages[c0] = stage

            hT = sb.tile([EP, BS], BF16)
            sd = sb.tile([P, NT], F32)   # sigmoid(pos dot)

            # --- per-tile: gathers -> h -> hT column block ---
            hsums = []
            for t in range(NT):
                hsum = gp.tile([P, E], F32, tag="hsum")
                for c in range(CTX):
                    g = gat.tile([P, E], F32, tag="gather")
                    nc.gpsimd.indirect_dma_start(
                        out=g[:], out_offset=None, in_=embv[:],
                        in_offset=bass.IndirectOffsetOnAxis(
                            ap=x_t[:, t * CTX + c: t * CTX + c + 1], axis=0))
                    if c == 0:
                        nc.vector.tensor_copy(hsum[:], g[:])
                    else:
                        nc.vector.tensor_add(hsum[:], hsum[:], g[:])
                nc.vector.tensor_scalar_mul(hsum[:], hsum[:], 1.0 / CTX)
                hsums.append(hsum)

                # bf16 cast, then transpose into hT via DVE 32x32 block
                # transposes (the xbar DMA-transpose serializes against all
                # in-flight DMA traffic, stalling ~18us; DVE doesn't)
                hbf = gp.tile([P, EP], BF16, tag="hbf")
                nc.vector.memset(hbf[:, E:EP], 0.0)
                nc.vector.tensor_copy(hbf[:, :E], hsum[:])
                for bi in range(P // 32):
                    for bj in range(EP // 32):
                        nc.vector.transpose(
                            out=hT[bj * 32:(bj + 1) * 32,
                                   t * P + bi * 32: t * P + (bi + 1) * 32],
                            in_=hbf[bi * 32:(bi + 1) * 32,
                                    bj * 32:(bj + 1) * 32])
                early = chunks[:1] if t == 0 else chunks[1:NPRE]
                for (c0, cn) in early:
                    nc.vector.tensor_copy(ut_b[:, c0:c0 + cn],
                                          stages[c0][:, :cn])

            # positive path dots (sd ACTIVATEs deferred to the end so they
            # don't block the ScalarE queue mid-kernel)
            dfull = sb.tile([P, NT], F32)
            for t in range(NT):
                uy = gat.tile([P, E], F32, tag="gather")
                nc.gpsimd.indirect_dma_start(
                    out=uy[:], out_offset=None, in_=embu[:],
                    in_offset=bass.IndirectOffsetOnAxis(
                        ap=y_t[:, t:t + 1], axis=0))
                prod = gp.tile([P, E], F32, tag="prod")
                nc.vector.tensor_mul(prod[:], uy[:], hsums[t][:])
                nc.vector.tensor_reduce(dfull[:, t:t + 1], prod[:],
                                        axis=mybir.AxisListType.X,
                                        op=mybir.AluOpType.add)

            # remaining ut chunks: SWDGE cast-during-DMA straight into the
            # bf16 table (halves the SBUF-write bytes; no staging, no DVE
            # casts). On the gpsimd queue these naturally wait out the
            # gathers, so they don't steal gather bandwidth.
            for (c0, cn) in chunks[NPRE:]:
                nc.gpsimd.dma_start(out=ut_b[:, c0:c0 + cn],
                                    in_=ut_in[:, c0:c0 + cn])

            # --- main loop: scores -> sigmoid -> per-row full-vocab sums ---
            S_part = sb.tile([P, NT], F32)
            sig_scr = sb.tile([P, GROUP], BF16)
            groups = [(i * GROUP, GROUP) for i in range(NFULL)]
            if TAIL:
                groups.append((NFULL * GROUP, TAIL))
            accs = [sb.tile([P, NG], F32, tag=f"acc{t}", name=f"acc{t}")
                    for t in range(NT)]
            # group-major order so each ut chunk is consumed by both batch
            # tiles back-to-back (halves the required stream bandwidth).
            # Tile 1's first two groups are deferred until its hT is ready.
            T0F = 8   # tile-0 groups run solo while tile-1's hT finishes
            sched = [(g, 0) for g in range(T0F)]
            sched += [x for g in range(T0F, NG)
                      for x in ((g, 0), (g - T0F, 1))]
            sched += [(g, 1) for g in range(NG - T0F, NG)]
            with tc.tile_pool(name="mm_psum", bufs=2, space="PSUM") as mmp:
                for (gi, t) in sched:
                    v0, vn = groups[gi]
                    lhsT = hT[:E, t * P:(t + 1) * P]
                    pg = mmp.tile([P, GROUP], F32)
                    for n0 in range(0, vn, MMN):
                        nn = min(MMN, vn - n0)
                        nc.tensor.matmul(
                            pg[:, n0:n0 + nn], lhsT,
                            ut_b[:, v0 + n0: v0 + n0 + nn],
                            start=True, stop=True)
                    nc.scalar.activation(
                        sig_scr[:, :vn], pg[:, :vn],
                        mybir.ActivationFunctionType.Sigmoid,
                        scale=-1.0, accum_out=accs[t][:, gi:gi + 1])
                # deferred positive-path sigmoids
                for t in range(NT):
                    nc.scalar.activation(sd[:, t:t + 1], dfull[:, t:t + 1],
                                         mybir.ActivationFunctionType.Sigmoid)
                for t in range(NT):
                    nc.vector.tensor_reduce(S_part[:, t:t + 1], accs[t][:],
                                            axis=mybir.AxisListType.X,
                                            op=mybir.AluOpType.add)

            # --- final: partial = sum_own_b ln(S_b / sd_b); AllReduce ---
            Gr = sb.tile([P, NT], F32)
            nc.vector.reciprocal(Gr[:], sd[:])
            R = sb.tile([P, NT], F32)
            nc.vector.tensor_mul(R[:], S_part[:], Gr[:])
            L = sb.tile([P, NT], F32)
            nc.scalar.activation(L[:], R[:], mybir.ActivationFunctionType.Ln)
            Lr = sb.tile([P, 1], F32)
            nc.vector.tensor_reduce(Lr[:], L[:], axis=mybir.AxisListType.X,
                                    op=mybir.AluOpType.add)
            ones = sb.tile([P, 1], F32)
            nc.vector.memset(ones[:], 1.0)
            with tc.tile_pool(name="fin_psum", bufs=1, space="PSUM") as fpp:
                lp = fpp.tile([1, 1], F32)
                nc.tensor.matmul(lp[:], ones[:], Lr[:], start=True, stop=True)
                ls = sb.tile([1, 1], F32)
                nc.scalar.mul(ls[:], lp[:], 1.0 / B)
                nc.sync.dma_start(out=loss_out[:], in_=ls[:])

    nc.compile()
    return nc


_nc_cache = None


def kernel(x_positive, y, emb_v, emb_u):
    global _nc_cache, _last_results
    x32 = np.ascontiguousarray(np.asarray(x_positive, dtype=np.int32))
    y32 = np.ascontiguousarray(np.asarray(y, dtype=np.int32)).reshape(B, 1)
    ev = np.ascontiguousarray(np.asarray(emb_v, dtype=np.float32))
    eu = np.ascontiguousarray(np.asarray(emb_u, dtype=np.float32))
    ut = np.ascontiguousarray(eu.T)

    if _nc_cache is None:
        _nc_cache = _build()
    nc = _nc_cache

    in_maps = []
    for c in range(N_CORES):
        in_maps.append({
            "x": x32[c * BS:(c + 1) * BS, :],
            "y": y32[c * BS:(c + 1) * BS, :],
            "emb_v": ev,
            "emb_u": eu,
            "ut": ut,
        })

    trace = bool(os.environ.get("BASS_TRACE"))
    res = run_bass_kernel_spmd(nc, in_maps, list(range(N_CORES)), trace=trace)
    _last_results = res
    loss = np.float32(sum(res.results[c]["loss"][0, 0]
                          for c in range(N_CORES)))
    return np.asarray(loss, dtype=np.float32).reshape(())
